# revision 1
# baseline (speedup 1.0000x reference)
"""Trainium2 Bass kernel for a dense transformer encoder block.

Sharding (8 cores): sequence-parallel. Core c handles batch b = c//4 and the
512-token query slice q0 = (c%4)*512. Each core computes K/V projections for
its full batch (duplicated across the 4 cores of a batch), attention for its
own queries over all 16 heads, then the FFN for its token slice. No
collectives; the host pre-transposes x / the mask and gathers the outputs.

On-chip layouts: projections and FFN mm1 run feature-major (contraction dim on
partitions); attention scores are computed k-major ([k_tokens | q] tiles) so
softmax-normalization folds into the AV matmul via an appended ones-column on
V; LayerNorms run token-major using bn_stats/bn_aggr. Matmuls use the fp32r
dtype (full PE rate for free dims >= 256).
"""

import sys
from contextlib import ExitStack

import numpy as np

for _p in ("/opt/trn_rl_repo", "/opt/pypackages"):
    if _p not in sys.path:
        sys.path.append(_p)

import ml_dtypes  # noqa: E402
import concourse.bass as bass  # noqa: E402
import concourse.tile as tile  # noqa: E402
from concourse import bacc, mybir  # noqa: E402
from concourse.masks import make_identity  # noqa: E402

F32 = mybir.dt.float32
F32R = mybir.dt.float32r
BF16 = mybir.dt.bfloat16
AF = mybir.ActivationFunctionType
ALU = mybir.AluOpType

P = 128
DH = 64            # head dim (fixed)
DFH = DH + 1       # head dim + ones column
LN_EPS = 1e-5
NEG = -1e30

FULL_CFG = dict(B=2, L=2048, D=1024, H=16, DFF=4096, NCORES=8)

# matmul dtype used for the big matmuls (fp32r = TF32-like, full PE rate)
MM_DT = F32R


def _mm(nc, out, lhsT, rhs, start, stop):
    nc.tensor.matmul(out, lhsT, rhs, start=start, stop=stop)


def build_bass(cfg):
    B, L, D, H, DFF = cfg["B"], cfg["L"], cfg["D"], cfg["H"], cfg["DFF"]
    NCORES = cfg["NCORES"]
    CPB = NCORES // B          # cores per batch
    TOK = L // CPB             # tokens per core
    KC = D // P                # contraction chunks over D
    KT = L // P                # key-token 128-chunks
    HPQ = min(4, H)            # heads per attention phase
    NQ = H // HPQ
    NTQ = TOK // P             # query-token 128-chunks per core
    DC = min(512, D)           # column chunk for D-wide outputs
    NDC = D // DC
    FQ = min(1024, DFF)        # w1 column block
    NFQ = DFF // FQ
    BNF = min(512, D)          # bn_stats subgroup
    NBN = D // BNF
    assert H * DH == D and TOK % P == 0 and L % 512 == 0

    nc = bacc.Bacc(None, target_bir_lowering=False, debug=False)
    with tile.TileContext(nc) as tc, ExitStack() as top, \
            nc.allow_low_precision(reason="fp32r operands for full-rate PE"):
        dram = top.enter_context(tc.tile_pool(name="dram", bufs=1, space="DRAM"))

        def din(name, shape, dtype=F32):
            return dram.tile(shape, dtype, kind="ExternalInput", name=name,
                             uniquify=False)

        xt_d = din("xt", [D, L], F32R)          # x[b].T
        xtq_d = din("xtq", [D, TOK], F32R)      # x[b, q0:q0+TOK].T
        xq_d = din("xq", [TOK, D])        # x[b, q0:q0+TOK]
        mt_d = din("mt", [L, TOK], BF16)  # additive mask, transposed slice
        wq_d = din("wq", [D, D], F32R)
        wk_d = din("wk", [D, D], F32R)
        wv_d = din("wv", [D, D], F32R)
        wo_d = din("wo", [D, D], F32R)
        w1_d = din("w1", [D, DFF], F32R)
        w2_d = din("w2", [DFF, D], F32R)
        bq_d = din("bq", [D])
        bk_d = din("bk", [D])
        vb_d = din("vb", [H * DFH])       # per-head [bv_h, 0]
        bo_d = din("bo", [D])
        b1_d = din("b1", [DFF])
        b2_d = din("b2", [D])
        g1_d = din("g1", [D])
        be1_d = din("be1", [D])
        g2_d = din("g2", [D])
        be2_d = din("be2", [D])
        out_d = dram.tile([TOK, D], F32, kind="ExternalOutput", name="out",
                          uniquify=False)
        ots_d = dram.tile([H, DH, TOK], F32R, kind="Internal", name="ots",
                          uniquify=False)  # attention-out bounce

        def bcast_row(src_ap):
            # DRAM [n] row -> AP broadcasting to P partitions
            return bass.AP(tensor=src_ap.tensor, offset=src_ap.offset,
                           ap=[[0, P]] + [list(a) for a in src_ap.ap])

        const = top.enter_context(tc.tile_pool(name="const", bufs=1))
        ident = const.tile([P, P], F32, name="ident")
        make_identity(nc, ident)
        ones65 = const.tile([DFH, DH], F32, name="ones65")
        nc.vector.memset(ones65[:], 1.0)
        eps_t = const.tile([P, 1], F32, name="eps_t")
        nc.vector.memset(eps_t[:], LN_EPS)
        bq_sb = const.tile([P, KC], F32, name="bq_sb")
        nc.sync.dma_start(out=bq_sb, in_=bq_d[:].rearrange("(c p) -> p c", p=P))
        bk_sb = const.tile([P, KC], F32, name="bk_sb")
        nc.sync.dma_start(out=bk_sb, in_=bk_d[:].rearrange("(c p) -> p c", p=P))
        b1_sb = const.tile([P, DFF // P], F32, name="b1_sb")
        nc.sync.dma_start(out=b1_sb, in_=b1_d[:].rearrange("(c p) -> p c", p=P))
        ones_c = const.tile([P, min(4, H), 1], F32, name="ones_c")
        nc.vector.memset(ones_c[:], 1.0)
        vb_bc = const.tile([P, H, DFH], F32, name="vb_bc")
        nc.sync.dma_start(
            out=vb_bc,
            in_=bcast_row(vb_d[:].rearrange("(h d) -> h d", d=DFH)))

        # ---------------- attention (projections + scores + AV) ------------
        with ExitStack() as attn:
            pa = attn.enter_context(tc.tile_pool(name="attn_sb", bufs=1))
            pw = attn.enter_context(tc.tile_pool(name="attn_w", bufs=2))
            pwork = attn.enter_context(tc.tile_pool(name="attn_wk", bufs=3))

            xt = []
            xt_r = xt_d[:].rearrange("(c p) l -> c p l", p=P)
            for kc in range(KC):
                t = pa.tile([P, L], F32R, name=f"xt{kc}", tag=f"xt{kc}")
                nc.sync.dma_start(out=t, in_=xt_r[kc])
                xt.append(t)
            xtq = []
            xtq_r = xtq_d[:].rearrange("(c p) l -> c p l", p=P)
            for kc in range(KC):
                t = pa.tile([P, TOK], F32R, name=f"xtq{kc}", tag=f"xtq{kc}")
                nc.sync.dma_start(out=t, in_=xtq_r[kc])
                xtq.append(t)
            mt = pa.tile([P, KT, TOK], BF16, name="mt", tag="mt")
            nc.sync.dma_start(out=mt,
                              in_=mt_d[:].rearrange("(t p) q -> p t q", p=P))

            wq_r = wq_d[:].rearrange("(c p) n -> p c n", p=P)
            wk_r = wk_d[:].rearrange("(c p) n -> p c n", p=P)
            wv_r = wv_d[:].rearrange("(c p) n -> p c n", p=P)

            for hq in range(NQ):
                h0 = hq * HPQ
                cs, ce = h0 * DH, (h0 + HPQ) * DH
                wkq = pa.tile([P, KC, HPQ * DH], F32R, name="wkq", tag="wkq")
                nc.sync.dma_start(out=wkq, in_=wk_r[:, :, cs:ce])
                wvq = pa.tile([P, KC, HPQ * DH], F32R, name="wvq", tag="wvq")
                nc.sync.dma_start(out=wvq, in_=wv_r[:, :, cs:ce])

                ktq = [pa.tile([P, L], F32R, name=f"ktq{i}", tag=f"ktq{i}")
                       for i in range(HPQ // 2)]
                vaug = [pa.tile([P, HPQ, DFH], F32R, name=f"vaug{t}",
                                tag=f"vaug{t}") for t in range(KT)]
                for t in range(KT):
                    nc.vector.tensor_copy(vaug[t][:, :, DH:DFH], ones_c)
                qtq = [pa.tile([P, TOK], F32R, name=f"qtq{i}", tag=f"qtq{i}")
                       for i in range(HPQ // 2)]

                with tc.tile_pool(name="psA", bufs=2, space="PSUM") as psA:
                    # K projection, feature-major, scaled by 1/8, +bias.
                    # One matmul per head-pair: the pair's 128 output features
                    # are contiguous weight columns.
                    for tg in range(L // 512):
                        kp = [psA.tile([P, 512], F32, name=f"kp{i}",
                                       tag=f"kp{i}") for i in range(HPQ // 2)]
                        for kc in range(KC):
                            for i in range(HPQ // 2):
                                _mm(nc, kp[i],
                                    wkq[:, kc, 2 * i * DH:(2 * i + 2) * DH],
                                    xt[kc][:, tg * 512:(tg + 1) * 512],
                                    start=(kc == 0), stop=(kc == KC - 1))
                        for i in range(HPQ // 2):
                            pc = (h0 + 2 * i) // 2
                            nc.vector.tensor_scalar(
                                ktq[i][:, tg * 512:(tg + 1) * 512], kp[i],
                                bk_sb[:, pc:pc + 1], 0.125,
                                ALU.add, ALU.mult)
                    # V projection, token-major, into [V|1] layout, +bias
                    for tg in range(L // 256):
                        vp = [psA.tile([P, HPQ * DH], F32, name=f"vp{j}",
                                       tag=f"vp{j}") for j in range(2)]
                        for kc in range(KC):
                            for j in range(2):
                                tci = tg * 2 + j
                                _mm(nc, vp[j],
                                    xt[kc][:, tci * P:(tci + 1) * P],
                                    wvq[:, kc, :],
                                    start=(kc == 0), stop=(kc == KC - 1))
                        for j in range(2):
                            tci = tg * 2 + j
                            nc.vector.tensor_tensor(
                                vaug[tci][:, :, 0:DH],
                                vp[j].rearrange("p (h d) -> p h d", d=DH),
                                vb_bc[:, h0:h0 + HPQ, 0:DH], ALU.add)

                with tc.tile_pool(name="psB", bufs=1, space="PSUM") as psB:
                    # Q projection for this quarter
                    qp = [psB.tile([P, TOK], F32, name=f"qp{i}", tag=f"qp{i}")
                          for i in range(HPQ // 2)]
                    for kc in range(KC):
                        wqt = pw.tile([P, HPQ * DH], F32R, name="wqt", tag="wqt")
                        nc.sync.dma_start(out=wqt, in_=wq_r[:, kc, cs:ce])
                        for i in range(HPQ // 2):
                            _mm(nc, qp[i],
                                wqt[:, 2 * i * DH:(2 * i + 2) * DH], xtq[kc],
                                start=(kc == 0), stop=(kc == KC - 1))
                    for i in range(HPQ // 2):
                        pc = (h0 + 2 * i) // 2
                        nc.vector.tensor_scalar(
                            qtq[i], qp[i], bq_sb[:, pc:pc + 1], None, ALU.add)

                    # scores + softmax + AV per head
                    for hl in range(HPQ):
                        h = h0 + hl
                        i, s = hl // 2, hl % 2
                        base = slice(s * DH, (s + 1) * DH)
                        otp = psB.tile([P, TOK], F32, name="otp", tag="otp",
                                       bufs=2)
                        for kt in range(KT):
                            sp = psB.tile([P, TOK], F32, name="sp", tag="sp",
                                          bufs=3)
                            _mm(nc, sp,
                                ktq[i][base, kt * P:(kt + 1) * P],
                                qtq[i][base, :], start=True, stop=True)
                            ss = pwork.tile([P, TOK], F32R, name="ss", tag="ss")
                            nc.vector.tensor_tensor(ss, sp, mt[:, kt, :],
                                                    ALU.add)
                            nc.scalar.activation(ss, ss, AF.Exp)
                            _mm(nc, otp[0:DFH, :], vaug[kt][:, hl, :], ss,
                                start=(kt == 0), stop=(kt == KT - 1))
                        rt = pwork.tile([DFH, TOK], F32, name="rt", tag="rt",
                                        bufs=2)
                        nc.vector.reciprocal(rt[DH:DFH, :], otp[DH:DFH, :])
                        rb = psB.tile([DH, TOK], F32, name="rb", tag="rb",
                                      bufs=1)
                        nc.tensor.matmul(rb, ones65[DH:DFH, :],
                                         rt[DH:DFH, :], start=True, stop=True)
                        # verifier forbids two PSUM inputs on tensor_tensor:
                        # bounce the broadcast through SBUF first
                        rbs = pwork.tile([DH, TOK], F32, name="rbs", tag="rbs",
                                         bufs=2)
                        nc.scalar.activation(rbs, rb, AF.Copy)
                        ot = pwork.tile([DH, TOK], F32R, name="ot", tag="ot",
                                        bufs=2)
                        nc.vector.tensor_tensor(ot, otp[0:DH, :], rbs, ALU.mult)
                        nc.sync.dma_start(out=ots_d[h], in_=ot)

        # ---------------- O-projection + LN1 + transpose + FFN --------------
        with ExitStack() as tail:
            pcd = tail.enter_context(tc.tile_pool(name="cd_sb", bufs=1))
            h_t = [pcd.tile([P, D], F32, name=f"h{t}", tag=f"h{t}")
                   for t in range(NTQ)]
            hT = [pcd.tile([P, TOK], F32R, name=f"hT{c}", tag=f"hT{c}")
                  for c in range(KC)]
            bo_bc = pcd.tile([P, D], F32, name="bo_bc")
            nc.sync.dma_start(out=bo_bc, in_=bcast_row(bo_d[:]))
            b2_bc = pcd.tile([P, D], F32, name="b2_bc")
            nc.sync.dma_start(out=b2_bc, in_=bcast_row(b2_d[:]))
            g1_bc = pcd.tile([P, D], F32, name="g1_bc")
            nc.sync.dma_start(out=g1_bc, in_=bcast_row(g1_d[:]))
            be1_bc = pcd.tile([P, D], F32, name="be1_bc")
            nc.sync.dma_start(out=be1_bc, in_=bcast_row(be1_d[:]))
            g2_bc = pcd.tile([P, D], F32, name="g2_bc")
            nc.sync.dma_start(out=g2_bc, in_=bcast_row(g2_d[:]))
            be2_bc = pcd.tile([P, D], F32, name="be2_bc")
            nc.sync.dma_start(out=be2_bc, in_=bcast_row(be2_d[:]))

            def layernorm(x_tile, g_bc, b_bc, wk):
                st = wk.tile([P, NBN, 6], F32, name="lnst", tag="lnst")
                xv = x_tile.rearrange("p (s f) -> p s f", f=BNF)
                for sg in range(NBN):
                    nc.vector.bn_stats(out=st[:, sg, :], in_=xv[:, sg, :])
                mv = wk.tile([P, 2], F32, name="lnmv", tag="lnmv")
                nc.vector.bn_aggr(out=mv, in_=st)
                sq = wk.tile([P, 1], F32, name="lnsq", tag="lnsq")
                nc.scalar.activation(sq, mv[:, 1:2], AF.Sqrt, bias=eps_t)
                nc.vector.reciprocal(sq, sq)
                nc.vector.tensor_scalar(x_tile, x_tile, mv[:, 0:1], sq,
                                        ALU.subtract, ALU.mult)
                nc.vector.tensor_tensor(x_tile, x_tile, g_bc, ALU.mult)
                nc.vector.tensor_tensor(x_tile, x_tile, b_bc, ALU.add)

            with ExitStack() as cph:
                pcw = cph.enter_context(tc.tile_pool(name="c_w", bufs=3))
                pcwk = cph.enter_context(tc.tile_pool(name="c_wk", bufs=3))
                pcp = cph.enter_context(tc.tile_pool(name="c_ps", bufs=1,
                                                     space="PSUM"))
                xq = []
                xq_r = xq_d[:].rearrange("(t p) d -> t p d", p=P)
                for t in range(NTQ):
                    tl = pcwk.tile([P, D], F32, name=f"xq{t}", tag=f"xq{t}",
                                   bufs=1)
                    nc.sync.dma_start(out=tl, in_=xq_r[t])
                    xq.append(tl)
                oT = []
                for h in range(H):
                    tl = pcwk.tile([DH, TOK], F32R, name=f"oT{h}", tag=f"oT{h}",
                                   bufs=1)
                    nc.sync.dma_start(out=tl, in_=ots_d[h])
                    oT.append(tl)
                for dc in range(NDC):
                    op = [pcp.tile([P, DC], F32, name=f"op{t}", tag=f"op{t}")
                          for t in range(NTQ)]
                    for h in range(H):
                        wot = pcw.tile([DH, DC], F32R, name="wot", tag="wot")
                        nc.sync.dma_start(
                            out=wot,
                            in_=wo_d[h * DH:(h + 1) * DH, dc * DC:(dc + 1) * DC])
                        for t in range(NTQ):
                            _mm(nc, op[t], oT[h][:, t * P:(t + 1) * P], wot,
                                start=(h == 0), stop=(h == H - 1))
                    for t in range(NTQ):
                        dsl = slice(dc * DC, (dc + 1) * DC)
                        nc.vector.tensor_tensor(h_t[t][:, dsl], op[t],
                                                xq[t][:, dsl], ALU.add)
                        nc.vector.tensor_tensor(h_t[t][:, dsl], h_t[t][:, dsl],
                                                bo_bc[:, dsl], ALU.add)
                for t in range(NTQ):
                    layernorm(h_t[t], g1_bc, be1_bc, pcwk)
                for t in range(NTQ):
                    for c in range(KC):
                        tp = pcp.tile([P, P], F32, name="tpp", tag="tpp",
                                      bufs=2)
                        nc.tensor.transpose(tp, h_t[t][:, c * P:(c + 1) * P],
                                            ident)
                        nc.scalar.activation(hT[c][:, t * P:(t + 1) * P], tp,
                                             AF.Copy)

            with ExitStack() as dph:
                pdw = dph.enter_context(tc.tile_pool(name="d_w", bufs=1))
                pdw2 = dph.enter_context(tc.tile_pool(name="d_w2", bufs=3))
                pdp = dph.enter_context(tc.tile_pool(name="d_ps", bufs=1,
                                                     space="PSUM"))
                f1 = [pdw.tile([P, TOK], F32R, name=f"f1_{ff}", tag=f"f1_{ff}")
                      for ff in range(DFF // P)]
                for fq in range(NFQ):
                    w1t = [pdw.tile([P, FQ], F32R, name=f"w1t{kc}",
                                    tag=f"w1t{kc}", bufs=1) for kc in range(KC)]
                    for kc in range(KC):
                        nc.sync.dma_start(
                            out=w1t[kc],
                            in_=w1_d[kc * P:(kc + 1) * P,
                                     fq * FQ:(fq + 1) * FQ])
                    for ffl in range(FQ // P):
                        ff = fq * (FQ // P) + ffl
                        fp = pdp.tile([P, TOK], F32, name="fp", tag="fp",
                                      bufs=2)
                        for kc in range(KC):
                            _mm(nc, fp, w1t[kc][:, ffl * P:(ffl + 1) * P],
                                hT[kc], start=(kc == 0), stop=(kc == KC - 1))
                        nc.scalar.activation(f1[ff], fp, AF.Relu,
                                             bias=b1_sb[:, ff:ff + 1])
                f2 = [pdw.tile([P, D], F32, name=f"f2_{t}", tag=f"f2_{t}")
                      for t in range(NTQ)]
                for dc in range(NDC):
                    g2p = [pdp.tile([P, DC], F32, name=f"g2p{t}",
                                    tag=f"g2p{t}") for t in range(NTQ)]
                    for kc2 in range(DFF // P):
                        w2t = pdw2.tile([P, DC], F32R, name="w2t", tag="w2t")
                        nc.sync.dma_start(
                            out=w2t,
                            in_=w2_d[kc2 * P:(kc2 + 1) * P,
                                     dc * DC:(dc + 1) * DC])
                        for t in range(NTQ):
                            _mm(nc, g2p[t], f1[kc2][:, t * P:(t + 1) * P], w2t,
                                start=(kc2 == 0), stop=(kc2 == DFF // P - 1))
                    for t in range(NTQ):
                        dsl = slice(dc * DC, (dc + 1) * DC)
                        nc.vector.tensor_tensor(f2[t][:, dsl], g2p[t],
                                                h_t[t][:, dsl], ALU.add)
                        nc.vector.tensor_tensor(f2[t][:, dsl], f2[t][:, dsl],
                                                b2_bc[:, dsl], ALU.add)
                for t in range(NTQ):
                    layernorm(f2[t], g2_bc, be2_bc, pdw2)
                    nc.sync.dma_start(out=out_d[t * P:(t + 1) * P, :],
                                      in_=f2[t])

    nc.compile()
    return nc


def make_in_maps(cfg, inp):
    """Build per-core input dicts from full (host) inputs."""
    B, L, D, H = cfg["B"], cfg["L"], cfg["D"], cfg["H"]
    NCORES = cfg["NCORES"]
    CPB = NCORES // B
    TOK = L // CPB
    f32 = np.float32
    x = np.asarray(inp["x"], f32)
    mask = np.asarray(inp["mask"], bool)
    w = {k: np.asarray(inp[k], f32) for k in
         ("wq", "bq", "wk", "bk", "wv", "bv", "wo", "bo", "w1", "b1",
          "w2", "b2", "ln1_g", "ln1_b", "ln2_g", "ln2_b")}
    vb = np.zeros((H, DFH), f32)
    vb[:, 0:DH] = w["bv"].reshape(H, DH)
    shared = dict(wq=w["wq"], wk=w["wk"], wv=w["wv"], wo=w["wo"],
                  w1=w["w1"], w2=w["w2"], bq=w["bq"], bk=w["bk"],
                  vb=vb.reshape(-1), bo=w["bo"], b1=w["b1"], b2=w["b2"],
                  g1=w["ln1_g"], be1=w["ln1_b"], g2=w["ln2_g"],
                  be2=w["ln2_b"])
    shared = {k: np.ascontiguousarray(v) for k, v in shared.items()}
    in_maps = []
    for c in range(NCORES):
        b, q0 = c // CPB, (c % CPB) * TOK
        xb = x[b]
        mt = np.where(mask[b, q0:q0 + TOK, :].T, f32(NEG), f32(0.0))
        m = dict(shared)
        m["xt"] = np.ascontiguousarray(xb.T)
        m["xtq"] = np.ascontiguousarray(xb[q0:q0 + TOK].T)
        m["xq"] = np.ascontiguousarray(xb[q0:q0 + TOK])
        m["mt"] = mt.astype(ml_dtypes.bfloat16)
        in_maps.append(m)
    return in_maps


_NC_CACHE = {}
TRACE = False
LAST_RESULTS = None


def _get_nc(key, cfg):
    if key not in _NC_CACHE:
        _NC_CACHE[key] = build_bass(cfg)
    return _NC_CACHE[key]


def kernel(**inputs):
    global LAST_RESULTS
    from concourse.bass_utils import run_bass_kernel_spmd

    cfg = FULL_CFG
    B, L, D = cfg["B"], cfg["L"], cfg["D"]
    NCORES = cfg["NCORES"]
    CPB = NCORES // B
    TOK = L // CPB
    nc = _get_nc("full", cfg)
    in_maps = make_in_maps(cfg, inputs)
    res = run_bass_kernel_spmd(nc, in_maps, core_ids=list(range(NCORES)),
                               trace=TRACE)
    LAST_RESULTS = res
    out = np.empty((B, L, D), np.float32)
    for c in range(NCORES):
        b, q0 = c // CPB, (c % CPB) * TOK
        out[b, q0:q0 + TOK] = res.results[c]["out"]
    return out

